# revision 15
# baseline (speedup 1.0000x reference)
"""Trainium2 Bass kernel for nn_Cross_attention_multi (sparse_attention).

Pipeline (8 NeuronCores, SPMD, one NEFF):
  Stage A  - 3D conv (SAME, 3x3x3) spatially sharded: each core convolves a
             6-row h-strip for all 32 channels of both x and y. bf16 matmuls
             with K=96 (ci x kh) and M=128 (4 output d-slices x 32 channels)
             accumulate 3 kw-taps per input d-slab into f32 PSUM; the kd tap
             is absorbed into the M-packing (each input slab feeds the 3
             output d's that need it with the right kd weights).
  AllToAll - one collective per tensor (x's overlaps y's conv) redistributes
             conv output (bf16) from spatial shards to channel shards in a
             patch-major layout.
  Stage B  - per channel: gather t^T [81, 1024] (patch dim on partitions),
             res_trans = W1/W2 matmuls + leaky relu(0.2), attention
             a^T.T @ b^T in [128, 512] PSUM tiles, streamed to HBM as f32.
"""

import sys

sys.path.insert(0, "/opt/trn_rl_repo")

import numpy as np
import ml_dtypes

import concourse.bass as bass
import concourse.bacc as bacc
import concourse.mybir as mybir
import concourse.tile as tile
from concourse import bass_utils

N_CORES = 8
C, D, H, W = 32, 36, 48, 48
P = 9
L = 1024
F32 = mybir.dt.float32
BF16 = mybir.dt.bfloat16
BF_NP = ml_dtypes.bfloat16


def build_program(n_iters=1, phases="abc"):
    nc = bacc.Bacc(
        "TRN2", target_bir_lowering=False, debug=False, num_devices=N_CORES
    )

    xs = nc.dram_tensor("xs", [C, D, 8, 50], BF16, kind="ExternalInput")
    ys = nc.dram_tensor("ys", [C, D, 8, 50], BF16, kind="ExternalInput")
    # [rel_d(6), kw(3), kh*32+ci (96), 32*dd+co (128)]
    lwx = nc.dram_tensor("lwx", [6, 3, 97, 128], BF16, kind="ExternalInput")
    lwy = nc.dram_tensor("lwy", [6, 3, 97, 128], BF16, kind="ExternalInput")
    ones1 = nc.dram_tensor("ones1", [1, D * 300], BF16, kind="ExternalInput")
    w1ta = nc.dram_tensor("w1ta", [81, 81], BF16, kind="ExternalInput")
    w1tb = nc.dram_tensor("w1tb", [81, 81], BF16, kind="ExternalInput")
    w2t1 = nc.dram_tensor("w2t1", [81, 81], BF16, kind="ExternalInput")
    w2t2 = nc.dram_tensor("w2t2", [81, 81], BF16, kind="ExternalInput")
    att = nc.dram_tensor("att", [4, L, L], F32, kind="ExternalOutput")

    Ident = mybir.ActivationFunctionType.Identity
    Copy = mybir.ActivationFunctionType.Copy
    mult = mybir.AluOpType.mult
    amax = mybir.AluOpType.max

    with tile.TileContext(nc) as tc:
        # [shard core][c_lo][p81 = pd*9+pw][ld*32 + lhw_local], per tensor
        with tc.tile_pool(name="dram", bufs=1, space="DRAM") as dram:
            a2a_in = [dram.tile([N_CORES, 4, 81, 128], BF16, name=f"a2ai{t}") for t in range(2)]
            a2a_out = [dram.tile([N_CORES, 4, 81, 128], BF16, name=f"a2ao{t}") for t in range(2)]

            for _it in range(n_iters):
                if _it:
                    tc.strict_bb_all_engine_barrier()
                # ------------ Stage A: conv (+ per-tensor AllToAll) ------
                with (
                    tc.tile_pool(name="slab", bufs=2) as slab_pool,
                    tc.tile_pool(name="wts", bufs=1) as wts_pool,
                    tc.tile_pool(name="stageA", bufs=3) as stage_pool,
                    tc.tile_pool(name="wtsB", bufs=1) as wtsB,
                    tc.tile_pool(name="sbB", bufs=2) as sbB,
                    tc.tile_pool(name="attst", bufs=3) as attst_pool,
                ):
                  if "a" in phases:
                    with (
                        tc.tile_pool(name="psumA", bufs=4, space="PSUM") as psumA,
                    ):
                        for tt, (src, lw_d) in enumerate(
                            [(xs, lwx), (ys, lwy)]
                        ):
                            lw = wts_pool.tile([97, 18, 128], BF16, tag=f"lw{tt}")
                            nc.sync.dma_start(
                                lw[:].rearrange("p (r k) m -> p r k m", k=3),
                                lw_d[:].transpose([2, 0, 1, 3]),
                            )

                            s_all = slab_pool.tile(
                                [97, D, 6, 50], BF16, tag="slab"
                            )
                            for kh in range(3):
                                nc.sync.dma_start(
                                    s_all[32 * kh : 32 * kh + 32],
                                    src[:, :, kh : kh + 6, :],
                                )
                            # ones row feeds the bias row of lw (K=97)
                            nc.sync.dma_start(
                                s_all[96:97].rearrange("p a b c -> p (a b c)"),
                                ones1[:],
                            )

                            stages = {}
                            for b in range(9):
                                rels = [
                                    r for r in range(6) if 0 <= 4 * b + r - 1 < D
                                ]
                                pt = psumA.tile([128, 288], F32, tag="pa")
                                n_mm = 3 * len(rels)
                                i = 0
                                for rel in rels:
                                    din = 4 * b + rel - 1
                                    for kw in range(3):
                                        nc.tensor.matmul(
                                            pt[:],
                                            lw[:, 3 * rel + kw, :],
                                            s_all[:, din, :, kw : kw + 48],
                                            start=(i == 0),
                                            stop=(i == n_mm - 1),
                                        )
                                        i += 1
                                for dd in range(4):
                                    d = 4 * b + dd
                                    ld, pd = d // 9, d % 9
                                    if ld not in stages:
                                        stages[ld] = stage_pool.tile(
                                            [32, 9, 9, 32], BF16,
                                            tag="st", name="stage",
                                        )
                                    # (lhw, pw)->(pw, lhw) swizzle, ->bf16
                                    dst = stages[ld][:, pd].transpose([0, 2, 1])
                                    srcp = pt[
                                        32 * dd : 32 * dd + 32
                                    ].rearrange("p (l w) -> p l w", w=9)
                                    if dd % 2 == 0:
                                        nc.scalar.activation(dst, srcp, Copy)
                                    else:
                                        nc.vector.tensor_copy(dst, srcp)
                                    if pd == 8:
                                        nc.sync.dma_start(
                                            a2a_in[tt][
                                                :, :, :, 32 * ld : 32 * ld + 32
                                            ],
                                            stages.pop(ld)[:],
                                        )
                            if "c" in phases:
                                nc.gpsimd.collective_compute(
                                    "AllToAll",
                                    mybir.AluOpType.bypass,
                                    replica_groups=[list(range(N_CORES))],
                                    ins=[a2a_in[tt].opt()],
                                    outs=[a2a_out[tt].opt()],
                                )

                  # ---------------- Stage B ----------------
                  if "b" in phases:
                    with (
                        tc.tile_pool(name="psumU", bufs=2, space="PSUM") as psumU,
                        tc.tile_pool(name="psumV", bufs=1, space="PSUM") as psumV,
                        tc.tile_pool(name="psumT", bufs=3, space="PSUM") as psumT,
                    ):
                        w1a_sb = wtsB.tile([81, 81], BF16, tag="w1a")
                        w1b_sb = wtsB.tile([81, 81], BF16, tag="w1b")
                        w2a_sb = wtsB.tile([81, 81], BF16, tag="w2a")
                        w2b_sb = wtsB.tile([81, 81], BF16, tag="w2b")
                        nc.sync.dma_start(w1a_sb[:], w1ta[:])
                        nc.sync.dma_start(w1b_sb[:], w1tb[:])
                        nc.sync.dma_start(w2a_sb[:], w2t1[:])
                        nc.sync.dma_start(w2b_sb[:], w2t2[:])

                        for c_lo in range(4):
                            aT = []
                            for tt in range(2):
                                tT = sbB.tile([81, L], BF16, tag=f"tT{tt}")
                                nc.sync.dma_start(
                                    tT[:].rearrange(
                                        "p (a i c) -> p a i c", i=8, c=32
                                    ),
                                    a2a_out[tt][:, c_lo].rearrange(
                                        "i p (a c) -> p a i c", c=32
                                    ),
                                )
                                a_sb = sbB.tile([81, L], BF16, tag=f"aT{tt}")
                                for nch in range(2):
                                    sl = slice(512 * nch, 512 * nch + 512)
                                    u2 = psumU.tile([81, 1024], F32, tag="u2")
                                    nc.tensor.matmul(
                                        u2[:, 0:512], w1a_sb[:], tT[:, sl],
                                        start=True, stop=True,
                                    )
                                    nc.tensor.matmul(
                                        u2[:, 512:1024], w1b_sb[:], tT[:, sl],
                                        start=True, stop=True,
                                    )
                                    u_sb = sbB.tile([81, 1024], BF16, tag="us")
                                    if nch == 0:
                                        nc.scalar.activation(u_sb[:], u2[:], Copy)
                                    else:
                                        nc.vector.tensor_copy(u_sb[:], u2[:])
                                    v = psumV.tile([81, 512], F32, tag="v")
                                    nc.tensor.matmul(
                                        v[:], w2a_sb[:], u_sb[:, 0:512],
                                        start=True, stop=False,
                                    )
                                    nc.tensor.matmul(
                                        v[:], w2b_sb[:], u_sb[:, 512:1024],
                                        start=False, stop=True,
                                    )
                                    # leaky relu: max(0.2 v, v); one PSUM input
                                    # per op -> t1 = 0.2 v (ACT scale-copy),
                                    # then max(t1, v) on DVE
                                    t1 = sbB.tile([81, 512], BF16, tag="t1")
                                    nc.scalar.activation(
                                        t1[:], v[:], Ident, scale=0.2
                                    )
                                    nc.vector.tensor_tensor(
                                        a_sb[:, sl], t1[:], v[:], amax
                                    )
                                aT.append(a_sb)
                            aTx, aTy = aT
                            for lc in range(8):
                                st = attst_pool.tile([128, L], F32, tag="attst")
                                for nch in range(2):
                                    pa = psumT.tile([128, 512], F32, tag="pt")
                                    nc.tensor.matmul(
                                        pa[:],
                                        aTx[:, 128 * lc : 128 * lc + 128],
                                        aTy[:, 512 * nch : 512 * nch + 512],
                                        start=True, stop=True,
                                    )
                                    dst = st[:, 512 * nch : 512 * nch + 512]
                                    if (2 * lc + nch) % 2 == 0:
                                        nc.scalar.activation(dst, pa[:], Copy)
                                    else:
                                        nc.vector.tensor_copy(dst, pa[:])
                                nc.sync.dma_start(
                                    att[c_lo, 128 * lc : 128 * lc + 128, :],
                                    st[:],
                                )

    nc.compile()
    return nc


def host_inputs(x, y, Wx, bx, Wy, by, W1, W2):
    x = np.asarray(x, np.float32)
    y = np.asarray(y, np.float32)
    Wx = np.asarray(Wx, np.float32)
    bx = np.asarray(bx, np.float32)
    Wy = np.asarray(Wy, np.float32)
    by = np.asarray(by, np.float32)
    W1 = np.asarray(W1, np.float32)
    W2 = np.asarray(W2, np.float32)

    def strips(x0):
        out = []
        for j in range(N_CORES):
            s = np.zeros((C, D, 8, 50), np.float32)
            r0, r1 = max(0, 6 * j - 1), min(48, 6 * j + 7)
            d0 = r0 - (6 * j - 1)
            s[:, :, d0 : d0 + (r1 - r0), 1:49] = x0[:, :, r0:r1, :]
            out.append(s.astype(BF_NP))
        return out

    def make_lw(Wc, bc):
        # lw[rel, kw, kh*32+ci, 32*dd+co] = Wc[co, ci, rel-dd, kh, kw];
        # row 96 carries the bias (fed by the ones-row of S) at kd=1, kw=1.
        lw = np.zeros((6, 3, 97, 128), np.float32)
        for rel in range(6):
            for dd in range(4):
                kd = rel - dd
                if 0 <= kd < 3:
                    # (co, ci, kh, kw) -> (kw, kh, ci, co)
                    blk = np.transpose(Wc[:, :, kd], (3, 2, 1, 0)).reshape(
                        3, 96, 32
                    )
                    lw[rel, :, :96, 32 * dd : 32 * dd + 32] = blk
                if kd == 1:
                    lw[rel, 1, 96, 32 * dd : 32 * dd + 32] = bc
        return lw.astype(BF_NP)

    xs_l, ys_l = strips(x[0]), strips(y[0])
    common = {
        "lwx": make_lw(Wx, bx),
        "lwy": make_lw(Wy, by),
        "ones1": np.ones((1, D * 300), BF_NP),
        "w1ta": np.ascontiguousarray(W1[:81].T).astype(BF_NP),
        "w1tb": np.ascontiguousarray(W1[81:].T).astype(BF_NP),
        "w2t1": np.ascontiguousarray((W2 / 9.0)[:, :81].T).astype(BF_NP),
        "w2t2": np.ascontiguousarray((W2 / 9.0)[:, 81:].T).astype(BF_NP),
    }
    return [
        {"xs": xs_l[j], "ys": ys_l[j], **common} for j in range(N_CORES)
    ]


_CACHED_NC = None


def get_program():
    global _CACHED_NC
    if _CACHED_NC is None:
        _CACHED_NC = build_program()
    return _CACHED_NC


def _probe_device():
    """Absorb a wedged-worker state left by a previous process: the first
    device op after a wedge fails and resets the worker; a retry succeeds."""
    import time

    import jax

    for _ in range(3):
        try:
            jax.block_until_ready(
                jax.jit(lambda a: a + 1)(np.zeros(8, np.float32))
            )
            return
        except Exception:
            time.sleep(2)


def kernel(x, y, Wx, bx, Wy, by, W1, W2):
    import time

    nc = get_program()
    in_maps = host_inputs(x, y, Wx, bx, Wy, by, W1, W2)
    _probe_device()
    last = None
    for _ in range(2):
        try:
            res = bass_utils.run_bass_kernel_spmd(
                nc, in_maps, core_ids=list(range(N_CORES))
            )
            break
        except Exception as e:
            last = e
            time.sleep(2)
    else:
        raise last
    out = np.concatenate([r["att"] for r in res.results], axis=0)[None]
    return out


# revision 17
# speedup vs baseline: 5.1011x; 5.1011x over previous
"""Trainium2 Bass kernel for nn_Cross_attention_multi (sparse_attention).

Pipeline (8 NeuronCores, SPMD, one NEFF):
  Stage A  - 3D conv (SAME, 3x3x3) spatially sharded: each core convolves a
             6-row h-strip for all 32 channels of both x and y. bf16 matmuls
             with K=96 (ci x kh) and M=128 (4 output d-slices x 32 channels)
             accumulate 3 kw-taps per input d-slab into f32 PSUM; the kd tap
             is absorbed into the M-packing (each input slab feeds the 3
             output d's that need it with the right kd weights).
  AllToAll - one collective per tensor (x's overlaps y's conv) redistributes
             conv output (bf16) from spatial shards to channel shards in a
             patch-major layout.
  Stage B  - per channel: gather t^T [81, 1024] (patch dim on partitions),
             res_trans = W1/W2 matmuls + leaky relu(0.2), attention
             a^T.T @ b^T in [128, 512] PSUM tiles, streamed to HBM as f32.
"""

import sys

sys.path.insert(0, "/opt/trn_rl_repo")

import numpy as np
import ml_dtypes

import concourse.bass as bass
import concourse.bacc as bacc
import concourse.mybir as mybir
import concourse.tile as tile
from concourse import bass_utils

N_CORES = 8
C, D, H, W = 32, 36, 48, 48
P = 9
L = 1024
F32 = mybir.dt.float32
BF16 = mybir.dt.bfloat16
BF_NP = ml_dtypes.bfloat16


def build_program(n_iters=1, phases="abc"):
    nc = bacc.Bacc(
        "TRN2", target_bir_lowering=False, debug=False, num_devices=N_CORES
    )

    xs = nc.dram_tensor("xs", [C, D, 8, 50], BF16, kind="ExternalInput")
    ys = nc.dram_tensor("ys", [C, D, 8, 50], BF16, kind="ExternalInput")
    # [rel_d(6), kw(3), kh*32+ci (96), 32*dd+co (128)]
    lwx = nc.dram_tensor("lwx", [6, 3, 97, 128], BF16, kind="ExternalInput")
    lwy = nc.dram_tensor("lwy", [6, 3, 97, 128], BF16, kind="ExternalInput")
    ones1 = nc.dram_tensor("ones1", [1, D * 300], BF16, kind="ExternalInput")
    w1ta = nc.dram_tensor("w1ta", [81, 81], BF16, kind="ExternalInput")
    w1tb = nc.dram_tensor("w1tb", [81, 81], BF16, kind="ExternalInput")
    w2t1 = nc.dram_tensor("w2t1", [81, 81], BF16, kind="ExternalInput")
    w2t2 = nc.dram_tensor("w2t2", [81, 81], BF16, kind="ExternalInput")
    att = nc.dram_tensor("att", [4, L, L], F32, kind="ExternalOutput")

    Ident = mybir.ActivationFunctionType.Identity
    Copy = mybir.ActivationFunctionType.Copy
    mult = mybir.AluOpType.mult
    amax = mybir.AluOpType.max

    with tile.TileContext(nc) as tc:
        # [shard core][c_lo][p81 = pd*9+pw][ld*32 + lhw_local], per tensor
        with tc.tile_pool(name="dram", bufs=1, space="DRAM") as dram:
            a2a_in = [dram.tile([N_CORES, 4, 81, 128], BF16, name=f"a2ai{t}") for t in range(2)]
            a2a_out = [dram.tile([N_CORES, 4, 81, 128], BF16, name=f"a2ao{t}") for t in range(2)]

            for _it in range(n_iters):
                if _it:
                    tc.strict_bb_all_engine_barrier()
                # ------------ Stage A: conv (+ per-tensor AllToAll) ------
                if "a" in phases:
                    with (
                        tc.tile_pool(name="slab", bufs=2) as slab_pool,
                        tc.tile_pool(name="wts", bufs=1) as wts_pool,
                        tc.tile_pool(name="stageA", bufs=4) as stage_pool,
                        tc.tile_pool(name="psumA", bufs=6, space="PSUM") as psumA,
                    ):
                        for tt, (src, lw_d) in enumerate(
                            [(xs, lwx), (ys, lwy)]
                        ):
                            lw = wts_pool.tile([97, 18, 128], BF16, tag=f"lw{tt}")
                            nc.sync.dma_start(
                                lw[:].rearrange("p (r k) m -> p r k m", k=3),
                                lw_d[:].transpose([2, 0, 1, 3]),
                            )

                            s_all = slab_pool.tile(
                                [97, D, 6, 50], BF16, tag="slab"
                            )
                            for kh in range(3):
                                nc.sync.dma_start(
                                    s_all[32 * kh : 32 * kh + 32],
                                    src[:, :, kh : kh + 6, :],
                                )
                            # ones row feeds the bias row of lw (K=97)
                            nc.sync.dma_start(
                                s_all[96:97].rearrange("p a b c -> p (a b c)"),
                                ones1[:],
                            )

                            stages = {}
                            for b in range(9):
                                rels = [
                                    r for r in range(6) if 0 <= 4 * b + r - 1 < D
                                ]
                                pt = psumA.tile([128, 288], F32, tag="pa")
                                n_mm = 3 * len(rels)
                                i = 0
                                for rel in rels:
                                    din = 4 * b + rel - 1
                                    for kw in range(3):
                                        nc.tensor.matmul(
                                            pt[:],
                                            lw[:, 3 * rel + kw, :],
                                            s_all[:, din, :, kw : kw + 48],
                                            start=(i == 0),
                                            stop=(i == n_mm - 1),
                                        )
                                        i += 1
                                for dd in range(4):
                                    d = 4 * b + dd
                                    ld, pd = d // 9, d % 9
                                    if ld not in stages:
                                        stages[ld] = stage_pool.tile(
                                            [32, 9, 9, 32], BF16,
                                            tag="st", name="stage",
                                        )
                                    # (lhw, pw)->(pw, lhw) swizzle, ->bf16
                                    dst = stages[ld][:, pd].transpose([0, 2, 1])
                                    srcp = pt[
                                        32 * dd : 32 * dd + 32
                                    ].rearrange("p (l w) -> p l w", w=9)
                                    if dd % 2 == 0:
                                        nc.scalar.activation(dst, srcp, Copy)
                                    else:
                                        nc.vector.tensor_copy(dst, srcp)
                                    if pd == 8:
                                        nc.sync.dma_start(
                                            a2a_in[tt][
                                                :, :, :, 32 * ld : 32 * ld + 32
                                            ],
                                            stages.pop(ld)[:],
                                        )
                            if "c" in phases:
                                nc.gpsimd.collective_compute(
                                    "AllToAll",
                                    mybir.AluOpType.bypass,
                                    replica_groups=[list(range(N_CORES))],
                                    ins=[a2a_in[tt].opt()],
                                    outs=[a2a_out[tt].opt()],
                                )

                # ---------------- Stage B ----------------
                if "b" in phases:
                    with (
                        tc.tile_pool(name="wtsB", bufs=1) as wtsB,
                        tc.tile_pool(name="sbB", bufs=2) as sbB,
                        tc.tile_pool(name="attst", bufs=4) as attst_pool,
                        tc.tile_pool(name="psumU", bufs=2, space="PSUM") as psumU,
                        tc.tile_pool(name="psumV", bufs=1, space="PSUM") as psumV,
                        tc.tile_pool(name="psumT", bufs=3, space="PSUM") as psumT,
                    ):
                        w1a_sb = wtsB.tile([81, 81], BF16, tag="w1a")
                        w1b_sb = wtsB.tile([81, 81], BF16, tag="w1b")
                        w2a_sb = wtsB.tile([81, 81], BF16, tag="w2a")
                        w2b_sb = wtsB.tile([81, 81], BF16, tag="w2b")
                        nc.sync.dma_start(w1a_sb[:], w1ta[:])
                        nc.sync.dma_start(w1b_sb[:], w1tb[:])
                        nc.sync.dma_start(w2a_sb[:], w2t1[:])
                        nc.sync.dma_start(w2b_sb[:], w2t2[:])

                        for c_lo in range(4):
                            aT = []
                            for tt in range(2):
                                tT = sbB.tile([81, L], BF16, tag=f"tT{tt}")
                                nc.sync.dma_start(
                                    tT[:].rearrange(
                                        "p (a i c) -> p a i c", i=8, c=32
                                    ),
                                    a2a_out[tt][:, c_lo].rearrange(
                                        "i p (a c) -> p a i c", c=32
                                    ),
                                )
                                a_sb = sbB.tile([81, L], BF16, tag=f"aT{tt}")
                                for nch in range(2):
                                    sl = slice(512 * nch, 512 * nch + 512)
                                    u2 = psumU.tile([81, 1024], F32, tag="u2")
                                    nc.tensor.matmul(
                                        u2[:, 0:512], w1a_sb[:], tT[:, sl],
                                        start=True, stop=True,
                                    )
                                    nc.tensor.matmul(
                                        u2[:, 512:1024], w1b_sb[:], tT[:, sl],
                                        start=True, stop=True,
                                    )
                                    u_sb = sbB.tile([81, 1024], BF16, tag="us")
                                    if nch == 0:
                                        nc.scalar.activation(u_sb[:], u2[:], Copy)
                                    else:
                                        nc.vector.tensor_copy(u_sb[:], u2[:])
                                    v = psumV.tile([81, 512], F32, tag="v")
                                    nc.tensor.matmul(
                                        v[:], w2a_sb[:], u_sb[:, 0:512],
                                        start=True, stop=False,
                                    )
                                    nc.tensor.matmul(
                                        v[:], w2b_sb[:], u_sb[:, 512:1024],
                                        start=False, stop=True,
                                    )
                                    # leaky relu: max(0.2 v, v); one PSUM input
                                    # per op -> t1 = 0.2 v (ACT scale-copy),
                                    # then max(t1, v) on DVE
                                    t1 = sbB.tile([81, 512], BF16, tag="t1")
                                    nc.scalar.activation(
                                        t1[:], v[:], Ident, scale=0.2
                                    )
                                    nc.vector.tensor_tensor(
                                        a_sb[:, sl], t1[:], v[:], amax
                                    )
                                aT.append(a_sb)
                            aTx, aTy = aT
                            for lc in range(8):
                                st = attst_pool.tile([128, L], F32, tag="attst")
                                for nch in range(2):
                                    pa = psumT.tile([128, 512], F32, tag="pt")
                                    nc.tensor.matmul(
                                        pa[:],
                                        aTx[:, 128 * lc : 128 * lc + 128],
                                        aTy[:, 512 * nch : 512 * nch + 512],
                                        start=True, stop=True,
                                    )
                                    dst = st[:, 512 * nch : 512 * nch + 512]
                                    if (2 * lc + nch) % 2 == 0:
                                        nc.scalar.activation(dst, pa[:], Copy)
                                    else:
                                        nc.vector.tensor_copy(dst, pa[:])
                                nc.scalar.dma_start(
                                    att[c_lo, 128 * lc : 128 * lc + 128, :],
                                    st[:],
                                )

    nc.compile()
    return nc


def host_inputs(x, y, Wx, bx, Wy, by, W1, W2):
    x = np.asarray(x, np.float32)
    y = np.asarray(y, np.float32)
    Wx = np.asarray(Wx, np.float32)
    bx = np.asarray(bx, np.float32)
    Wy = np.asarray(Wy, np.float32)
    by = np.asarray(by, np.float32)
    W1 = np.asarray(W1, np.float32)
    W2 = np.asarray(W2, np.float32)

    def strips(x0):
        out = []
        for j in range(N_CORES):
            s = np.zeros((C, D, 8, 50), np.float32)
            r0, r1 = max(0, 6 * j - 1), min(48, 6 * j + 7)
            d0 = r0 - (6 * j - 1)
            s[:, :, d0 : d0 + (r1 - r0), 1:49] = x0[:, :, r0:r1, :]
            out.append(s.astype(BF_NP))
        return out

    def make_lw(Wc, bc):
        # lw[rel, kw, kh*32+ci, 32*dd+co] = Wc[co, ci, rel-dd, kh, kw];
        # row 96 carries the bias (fed by the ones-row of S) at kd=1, kw=1.
        lw = np.zeros((6, 3, 97, 128), np.float32)
        for rel in range(6):
            for dd in range(4):
                kd = rel - dd
                if 0 <= kd < 3:
                    # (co, ci, kh, kw) -> (kw, kh, ci, co)
                    blk = np.transpose(Wc[:, :, kd], (3, 2, 1, 0)).reshape(
                        3, 96, 32
                    )
                    lw[rel, :, :96, 32 * dd : 32 * dd + 32] = blk
                if kd == 1:
                    lw[rel, 1, 96, 32 * dd : 32 * dd + 32] = bc
        return lw.astype(BF_NP)

    xs_l, ys_l = strips(x[0]), strips(y[0])
    common = {
        "lwx": make_lw(Wx, bx),
        "lwy": make_lw(Wy, by),
        "ones1": np.ones((1, D * 300), BF_NP),
        "w1ta": np.ascontiguousarray(W1[:81].T).astype(BF_NP),
        "w1tb": np.ascontiguousarray(W1[81:].T).astype(BF_NP),
        "w2t1": np.ascontiguousarray((W2 / 9.0)[:, :81].T).astype(BF_NP),
        "w2t2": np.ascontiguousarray((W2 / 9.0)[:, 81:].T).astype(BF_NP),
    }
    return [
        {"xs": xs_l[j], "ys": ys_l[j], **common} for j in range(N_CORES)
    ]


_CACHED_NC = None


def get_program():
    global _CACHED_NC
    if _CACHED_NC is None:
        _CACHED_NC = build_program()
    return _CACHED_NC


def _probe_device():
    """Absorb a wedged-worker state left by a previous process: the first
    device op after a wedge fails and resets the worker; a retry succeeds."""
    import time

    import jax

    for _ in range(3):
        try:
            jax.block_until_ready(
                jax.jit(lambda a: a + 1)(np.zeros(8, np.float32))
            )
            return
        except Exception:
            time.sleep(2)


def kernel(x, y, Wx, bx, Wy, by, W1, W2):
    import time

    nc = get_program()
    in_maps = host_inputs(x, y, Wx, bx, Wy, by, W1, W2)
    _probe_device()
    last = None
    for _ in range(2):
        try:
            res = bass_utils.run_bass_kernel_spmd(
                nc, in_maps, core_ids=list(range(N_CORES))
            )
            break
        except Exception as e:
            last = e
            time.sleep(2)
    else:
        raise last
    out = np.concatenate([r["att"] for r in res.results], axis=0)[None]
    return out
